# revision 5
# baseline (speedup 1.0000x reference)
"""Trainium2 Bass kernel for nn_Cross_Message (GNN message passing).

Strategy (8 NeuronCores, SPMD):
  - Host: relabel source nodes by degree (descending) into 392 groups of 128;
    deal groups round-robin to the 8 cores (49 groups each) so every core runs
    the same compile-time column schedule Ksched[i] = max slots needed at
    position i. Each node owns one SBUF partition of its group; its edges
    occupy that partition's column slots. This turns segment softmax +
    weighted segment-sum into per-partition ops with zero cross-partition
    communication and no all-reduce (each core owns disjoint output rows).
  - Device per group: indirect-DMA gather of raw X_h_2 rows (one [128]-row
    gather per column), fused dot / MAC on the vector engine
    (scalar_tensor_tensor with accum_out), squared norms on the scalar engine
    (Square with accumulate), softmax via one ACT exp with accumulate,
    gate = sigmoid via exp + reciprocal (single ACT table set),
    gate matmul on the tensor engine.
  - Host: inverse-permute the 8 per-core outputs into the full [N1, 128].

Self-contained: hardcodes problem shapes; imports only numpy + concourse.
"""
import os
import sys

import numpy as np

for _p in ("/opt/trn_rl_repo", "/root/.axon_site/_ro/trn_rl_repo"):
    if os.path.isdir(_p) and _p not in sys.path:
        sys.path.append(_p)

N1 = 50000
N2 = 50000
E = 640000
D = 128      # node feature dim
A = 64       # attr dim
P = 128      # partitions
NCORES = 8
G = 392      # groups (392*128 = 50176 >= N1)
GPC = G // NCORES
EPS = 1e-8
MASKNEG = -60.0
TINY = 1e-30

LAST_EXEC_NS = None


def _prep(X_h_1, X_h_2, X_n_1, cross_indices, W_gate):
    src = np.asarray(cross_indices[0], dtype=np.int64)
    dst = np.asarray(cross_indices[1], dtype=np.int64)
    X_h_1 = np.asarray(X_h_1, dtype=np.float32)
    X_h_2 = np.asarray(X_h_2, dtype=np.float32)
    X_n_1 = np.asarray(X_n_1, dtype=np.float32)
    W_gate = np.asarray(W_gate, dtype=np.float32)

    deg = np.bincount(src, minlength=N1).astype(np.int64)
    node_order = np.argsort(-deg, kind="stable")
    node_order_p = np.full(G * P, -1, dtype=np.int64)
    node_order_p[:N1] = node_order
    deg_p = np.where(node_order_p >= 0, deg[np.clip(node_order_p, 0, N1 - 1)], 0)

    Kg = deg_p.reshape(G, P).max(axis=1)
    Ksched = Kg.reshape(GPC, NCORES).max(axis=1).astype(np.int64)
    sumK = int(Ksched.sum())

    eorder = np.argsort(src, kind="stable")
    dst_sorted = dst[eorder]
    off = np.zeros(N1 + 1, dtype=np.int64)
    off[1:] = np.cumsum(deg)

    per_core = []
    for c in range(NCORES):
        idx_all = np.zeros((P, sumK), dtype=np.int32)
        mneg_all = np.full((P, sumK), MASKNEG, dtype=np.float32)
        x1t = np.zeros((P, GPC * D), dtype=np.float32)
        xnt = np.zeros((P, GPC * P), dtype=np.float32)
        koff = 0
        for i in range(GPC):
            g = i * NCORES + c
            K = int(Ksched[i])
            nodes = node_order_p[g * P:(g + 1) * P]
            degs = deg_p[g * P:(g + 1) * P]
            if K > 0:
                col = np.arange(K)[None, :]
                valid = col < degs[:, None]
                base = np.where(nodes >= 0, off[np.clip(nodes, 0, N1 - 1)], 0)
                epos = base[:, None] + col
                blk_idx = np.zeros((P, K), dtype=np.int32)
                blk_idx[valid] = dst_sorted[np.clip(epos, 0, E - 1)][valid].astype(np.int32)
                idx_all[:, koff:koff + K] = blk_idx
                mneg_all[:, koff:koff + K][valid] = 0.0
                koff += K
            vn = nodes >= 0
            x1t[:, i * D:(i + 1) * D][vn, :] = X_h_1[nodes[vn]]
            xnt[:A, i * P:(i + 1) * P][:, vn] = X_n_1[nodes[vn]].T
        per_core.append(dict(idx_all=idx_all, mneg_all=mneg_all,
                             x1t=x1t, xnt=xnt))

    wgt = np.zeros((P, P), dtype=np.float32)
    wgt[:A, :] = W_gate.T

    meta = dict(Ksched=tuple(int(k) for k in Ksched), node_order_p=node_order_p,
                deg=deg, x2t=X_h_2, wgt=wgt, sumK=sumK)
    return per_core, meta


def _build(Ksched, sumK):
    import concourse.bass as bass
    import concourse.mybir as mybir
    from concourse import bacc
    from concourse.tile import TileContext

    f32 = mybir.dt.float32
    i32 = mybir.dt.int32
    AF = mybir.ActivationFunctionType
    ALU = mybir.AluOpType

    nc = bacc.Bacc()
    x2t = nc.dram_tensor("x2t", [N2, D], f32, kind="ExternalInput")
    x1g = nc.dram_tensor("x1g", [P, GPC * D], f32, kind="ExternalInput")
    idxs = nc.dram_tensor("idxs", [P, max(sumK, 1)], i32, kind="ExternalInput")
    mnegs = nc.dram_tensor("mnegs", [P, max(sumK, 1)], f32, kind="ExternalInput")
    xnt = nc.dram_tensor("xnt", [P, GPC * P], f32, kind="ExternalInput")
    wgt = nc.dram_tensor("wgt", [P, P], f32, kind="ExternalInput")
    out = nc.dram_tensor("out", [GPC * P, D], f32, kind="ExternalOutput")

    with TileContext(nc) as tc:
        with (
            tc.tile_pool(name="const", bufs=1) as cp,
            tc.tile_pool(name="sb", bufs=4) as sb,
            tc.tile_pool(name="x2p", bufs=4) as x2p,
            tc.tile_pool(name="ps", bufs=2, space="PSUM") as ps,
        ):
            wgt_sb = cp.tile([P, P], f32)
            nc.sync.dma_start(out=wgt_sb[:], in_=wgt[:, :])
            neg1 = cp.tile([P, 1], f32)
            nc.vector.memset(neg1[:], -1.0)
            gates = cp.tile([P, GPC * P], f32)

            idx_all = cp.tile([P, max(sumK, 1)], i32)
            nc.sync.dma_start(out=idx_all[:], in_=idxs[:, :])
            mneg_all = cp.tile([P, max(sumK, 1)], f32)
            nc.sync.dma_start(out=mneg_all[:], in_=mnegs[:, :])
            x1_all = cp.tile([P, GPC * D], f32)
            nc.sync.dma_start(out=x1_all[:], in_=x1g[:, :])
            xnt_all = cp.tile([P, GPC * P], f32)
            nc.sync.dma_start(out=xnt_all[:], in_=xnt[:, :])

            # ---- software-pipelined main loop ----
            # per group i: gate unit (PE/ACT/DVE), then stage A (gather +
            # dot/nsq + softmax weights), then stage B of group i-1
            # (MACs + gated output). The one-group lag keeps every engine's
            # in-order stream from blocking on a cross-engine wait.
            state = {}

            def stage_b(j):
                K, x2_sb, ex, r = state.pop(j)
                aggU = sb.tile([P, D], f32, tag="aggU")
                nc.vector.tensor_scalar_mul(out=aggU[:], in0=x2_sb[:, 0:D],
                                            scalar1=ex[:, 0:1])
                for k in range(1, K):
                    nc.vector.scalar_tensor_tensor(
                        out=aggU[:], in0=x2_sb[:, k * D:(k + 1) * D],
                        scalar=ex[:, k:k + 1], in1=aggU[:],
                        op0=ALU.mult, op1=ALU.add)
                out_sb = sb.tile([P, D], f32, tag="outt")
                nc.vector.scalar_tensor_tensor(
                    out=out_sb[:], in0=aggU[:], scalar=r[:],
                    in1=gates[:, j * P:(j + 1) * P],
                    op0=ALU.mult, op1=ALU.mult)
                nc.sync.dma_start(out=out[j * P:(j + 1) * P, :], in_=out_sb[:])

            koff = 0
            prev = None
            for i in range(GPC):
                # gate unit i: gates = sigmoid(Xn @ Wg.T) = 1/(1+exp(-x))
                gps = ps.tile([P, P], f32, space="PSUM")
                nc.tensor.matmul(gps[:], lhsT=xnt_all[:, i * P:(i + 1) * P],
                                 rhs=wgt_sb[:], start=True, stop=True)
                ge = sb.tile([P, P], f32, tag="ge")
                nc.scalar.activation(out=ge[:], in_=gps[:], func=AF.Exp,
                                     bias=0.0, scale=-1.0)
                nc.vector.tensor_scalar_add(out=ge[:], in0=ge[:], scalar1=1.0)
                nc.vector.reciprocal(out=gates[:, i * P:(i + 1) * P], in_=ge[:])

                K = Ksched[i]
                if K == 0:
                    continue
                idx_sb = idx_all[:, koff:koff + K]
                mneg_sb = mneg_all[:, koff:koff + K]
                koff += K
                x1_sb = x1_all[:, i * D:(i + 1) * D]

                x2_sb = x2p.tile([P, K * D], f32, tag="x2")
                for k in range(K):
                    nc.gpsimd.indirect_dma_start(
                        out=x2_sb[:, k * D:(k + 1) * D],
                        out_offset=None,
                        in_=x2t[:],
                        in_offset=bass.IndirectOffsetOnAxis(
                            ap=idx_sb[:, k:k + 1], axis=0))

                scr = sb.tile([P, D], f32, tag="scr")
                nsq1 = sb.tile([P, 1], f32, tag="nsq1")
                nc.vector.scalar_tensor_tensor(
                    out=scr[:], in0=x1_sb, scalar=0.0, in1=x1_sb,
                    op0=ALU.bypass, op1=ALU.mult, accum_out=nsq1[:])
                nc.vector.tensor_scalar_max(out=nsq1[:], in0=nsq1[:],
                                            scalar1=float(EPS * EPS))
                l1 = sb.tile([P, 1], f32, tag="l1")
                nc.scalar.activation(out=l1[:], in_=nsq1[:], func=AF.Ln)

                dot = sb.tile([P, K], f32, tag="dot")
                nsq2 = sb.tile([P, K], f32, tag="nsq2")
                scr2 = sb.tile([P, D], f32, tag="scr2")
                for k in range(K):
                    x2k = x2_sb[:, k * D:(k + 1) * D]
                    nc.vector.scalar_tensor_tensor(
                        out=scr[:], in0=x2k, scalar=0.0, in1=x1_sb,
                        op0=ALU.bypass, op1=ALU.mult,
                        accum_out=dot[:, k:k + 1])
                    if k % 2 == 0:
                        nc.scalar.activation(
                            out=scr2[:], in_=x2k, func=AF.Square,
                            accum_out=nsq2[:, k:k + 1])
                    else:
                        nc.vector.scalar_tensor_tensor(
                            out=scr2[:], in0=x2k, scalar=0.0, in1=x2k,
                            op0=ALU.bypass, op1=ALU.mult,
                            accum_out=nsq2[:, k:k + 1])

                nc.vector.tensor_scalar_max(out=nsq2[:], in0=nsq2[:],
                                            scalar1=float(EPS * EPS))
                lsum = sb.tile([P, K], f32, tag="lsum")
                nc.scalar.activation(out=lsum[:], in_=nsq2[:], func=AF.Ln)
                nc.vector.tensor_scalar_add(out=lsum[:], in0=lsum[:],
                                            scalar1=l1[:, 0:1])
                rn12 = sb.tile([P, K], f32, tag="rn12")
                nc.scalar.activation(out=rn12[:], in_=lsum[:], func=AF.Exp,
                                     bias=0.0, scale=-0.5)
                sim = sb.tile([P, K], f32, tag="sim")
                nc.vector.tensor_tensor(out=sim[:], in0=dot[:], in1=rn12[:],
                                        op=ALU.mult)
                nc.vector.tensor_tensor(out=sim[:], in0=sim[:], in1=mneg_sb,
                                        op=ALU.add)
                ex = sb.tile([P, K], f32, tag="ex")
                S = sb.tile([P, 1], f32, tag="S")
                nc.scalar.activation(out=ex[:], in_=sim[:], func=AF.Exp,
                                     bias=neg1[:], scale=1.0, accum_out=S[:])
                nc.vector.tensor_scalar_add(out=S[:], in0=S[:],
                                            scalar1=float(TINY))
                r = sb.tile([P, 1], f32, tag="r")
                nc.vector.reciprocal(out=r[:], in_=S[:])

                state[i] = (K, x2_sb, ex, r)
                if prev is not None:
                    stage_b(prev)
                prev = i
            if prev is not None:
                stage_b(prev)
    nc.compile()
    return nc


def kernel(X_h_1, X_h_2, X_n_1, cross_indices, W_gate):
    global LAST_EXEC_NS
    from concourse.bass_utils import run_bass_kernel_spmd

    per_core, meta = _prep(X_h_1, X_h_2, X_n_1, cross_indices, W_gate)
    nc = _build(meta["Ksched"], meta["sumK"])

    x2t = np.ascontiguousarray(meta["x2t"])
    in_maps = []
    for c in range(NCORES):
        pc = per_core[c]
        in_maps.append(dict(x2t=x2t, x1g=pc["x1t"], idxs=pc["idx_all"],
                            mnegs=pc["mneg_all"], xnt=pc["xnt"],
                            wgt=meta["wgt"]))

    trace = bool(int(os.environ.get("BASS_KERNEL_TRACE", "0")))
    try:
        res = run_bass_kernel_spmd(nc, in_maps, list(range(NCORES)),
                                   trace=trace)
    except ModuleNotFoundError:
        res = run_bass_kernel_spmd(nc, in_maps, list(range(NCORES)),
                                   trace=False)
    LAST_EXEC_NS = res.exec_time_ns

    node_order_p = meta["node_order_p"]
    deg = meta["deg"]
    out_full = np.zeros((N1, D), dtype=np.float32)
    for c in range(NCORES):
        rows = res.results[c]["out"]
        for i in range(GPC):
            g = i * NCORES + c
            nodes = node_order_p[g * P:(g + 1) * P]
            vn = nodes >= 0
            out_full[nodes[vn]] = rows[i * P:(i + 1) * P][vn]
    out_full[deg == 0] = 0.0
    return out_full
